# revision 14
# baseline (speedup 1.0000x reference)
"""Trainium2 Bass kernel for nn_CustomConv2d_32538672234916.

out[b,o,h,w] = K - sum_{ci,kh,kw} exp(x_patch)*exp(w) + bias[o],  K = Cin*kh*kw = 576
i.e. out = (K + bias) - conv2d(exp(x) [1-padded], exp(weight), stride 1)

Sharding: data-parallel over batch B=16 across 8 cores (2 batches/core),
weights/bias replicated.

Per-core GEMM formulation (fp8 DoubleRow): SBUF holds exp(x) in fp8 three
times: region0 half0 (partitions 0-63) = padded exp(x) "E", region0 half1
(partitions 64-127) = E shifted down one padded row, region1 (free offset
+PAD_TOT) half0 = E shifted down two rows (region1 half1 is only ever
multiplied by phantom zero weights). Each 8-row output strip (512 px)
accumulates THREE MatmulPerfMode.DoubleRow fp8 matmuls (0.5 PE-cycles/row;
k-tile-dim stride = PAD_TOT, nested/non-overlapping -- the hw AP walker
rejects overlapping k-tile strides). Matmul g=kw covers the tap column:
  ktile0 -> taps (0,kw) [half0] + (1,kw) [half1]
  ktile1 -> taps (2,kw) [region1 half0] + phantom w=0 [region1 half1]
Tensor time ~= 16 strips * 3 * 256 cyc = 5.1us/core.

IO: x uploaded bf16 (1MB/core), exp runs bf16->fp8 on the Act engine, out
written bf16 (2MB/core) and widened to f32 on host. Epilogue (PSUM f32 *-1
+ (K+bias) -> bf16) alternates DVE / GpSimd so neither engine is the wall.
fp8 end-to-end rel err ~7e-3 vs the 2e-2 gate.
"""
import sys
sys.path.insert(0, '/opt/trn_rl_repo')
import numpy as np

B, CIN, H, W = 16, 64, 64, 64
COUT = 128
NCORES = 8
BL = B // NCORES          # batches per core
PAD_W = W + 2             # 66
PAD_TOT = PAD_W * (H + 2) # 66*66 = 4356
KSUM = float(CIN * 9)     # 576
ROWS_PER_TILE = 8
NTILES = H // ROWS_PER_TILE  # 8 strips per image
HH = H // 2

# taps covered: [(half0 tap, half1 tap), ...] per (group=kw, ktile);
# ktile0 reads region0 = (E, E+1row), ktile1 reads region1 = (E+2rows, any)
# at k-tile-dim stride PAD_TOT; None = phantom zero weight.
GROUP_TAPS = [
    [((0, 0), (1, 0)), ((2, 0), None)],
    [((0, 1), (1, 1)), ((2, 1), None)],
    [((0, 2), (1, 2)), ((2, 2), None)],
]

_CACHE = {}


def _build(reps=1):
    from concourse import bacc, mybir
    from concourse.tile import TileContext

    f32 = mybir.dt.float32
    bf16 = mybir.dt.bfloat16
    f8 = mybir.dt.float8e4
    Exp = mybir.ActivationFunctionType.Exp
    DR = mybir.MatmulPerfMode.DoubleRow

    nc = bacc.Bacc("TRN2", target_bir_lowering=False, debug=False)
    x_d = nc.dram_tensor("x", [BL, CIN, H, W], bf16, kind="ExternalInput")
    wdr_d = nc.dram_tensor("wdr", [128, 3 * 2 * COUT], f8, kind="ExternalInput")
    bvec_d = nc.dram_tensor("bvec", [COUT, 1], f32, kind="ExternalInput")
    out_d = nc.dram_tensor("out", [BL, COUT, H, W], bf16, kind="ExternalOutput")
    x_ap = x_d.ap()
    out_ap = out_d.ap()

    with TileContext(nc) as tc:
        with tc.tile_pool(name="consts", bufs=1) as consts, \
             tc.tile_pool(name="xp", bufs=2) as xp, \
             tc.tile_pool(name="ep", bufs=2) as ep, \
             tc.tile_pool(name="rp", bufs=4) as rp, \
             tc.tile_pool(name="pp", bufs=1, space="PSUM") as pp:
            wdr_t = consts.tile([128, 3 * 2 * COUT], f8)
            bv_t = consts.tile([COUT, 1], f32)
            wdr4 = wdr_t.rearrange("p (g k m) -> p g k m", g=3, k=2)

            def load_consts():
                nc.sync.dma_start(wdr_t[:], wdr_d.ap())
                nc.sync.dma_start(bv_t[:], bvec_d.ap())

            # out-DMA granularity: OUTG strips per transfer. Fewer DMA
            # instructions = less (shared) descriptor-gen serialization.
            OUTG = 4
            SPB = ROWS_PER_TILE * W
            epi_state = {}

            def epilogue(i, b, t, pt):
                # Evacuate a PAIR of strips (one PSUM tile spanning 2 banks)
                # per instruction: PSUM f32 * -1 + (K+bias) -> bf16. GPSIMD
                # can't read PSUM on this target, so pairs split between DVE
                # (7/8) and the Act engine (1/8; Identity shares the loaded
                # act table with Exp). Fires on the pair's second strip.
                if t % 2 == 0:
                    return
                slot = t % OUTG
                if slot == 1:
                    epi_state["res"] = rp.tile([COUT, OUTG * SPB], bf16,
                                               tag="res", name=f"res_{b}_{t}")
                res = epi_state["res"]
                dst = res[:, (slot - 1) * SPB:(slot + 1) * SPB]
                on_act = (t == 3 and i % 2 == 0)
                if on_act:
                    nc.scalar.activation(dst, pt[:],
                                         mybir.ActivationFunctionType.Identity,
                                         bias=bv_t[:], scale=-1.0)
                else:
                    nc.vector.tensor_scalar(dst, pt[:], -1.0, bv_t[:],
                                            mybir.AluOpType.mult,
                                            mybir.AluOpType.add)
                if slot == OUTG - 1:
                    # out-DMA via the (otherwise idle) gpsimd SWDGE ring:
                    # off the SP queue and off the shared HWDGE
                    t0 = t - (OUTG - 1)
                    nc.gpsimd.dma_start(
                        out_ap[b][:, t0 * ROWS_PER_TILE:(t + 1) * ROWS_PER_TILE, :],
                        res[:])

            def make_rhs(et, t, kw):
                # [p, 2(k-tile, stride PAD_TOT), 8 rows, 64 cols] — nested
                # non-overlapping strides (hw rejects overlapping k-tile APs)
                e4 = et.rearrange("p (r h w) -> p r h w", r=2, w=PAD_W)
                h0 = t * ROWS_PER_TILE
                return e4[0:128, :, h0:h0 + ROWS_PER_TILE, kw:kw + W]

            ets = {}
            for i, b in enumerate([b for _ in range(reps) for b in range(BL)]):
                xt = xp.tile([CIN, H * W], bf16, tag="xt", name=f"xt_{i}")
                nc.sync.dma_start(xt[:], x_ap[b].rearrange("c h w -> c (h w)"))
                if i == 0:
                    # after the first x DMA so x-data flows immediately
                    load_consts()
                xt3 = xt.rearrange("p (h w) -> p h w", w=W)
                bufslot = i % 2
                if bufslot not in ets:
                    # pad cells are written once per buffer and persist
                    # (later iterations only rewrite the interiors)
                    et = ep.tile([128, 2 * PAD_TOT], f8, tag="et",
                                 name=f"et_{bufslot}")
                    ets[bufslot] = et
                    e3 = et.rearrange("p (h w) -> p h w", w=PAD_W)
                    nc.gpsimd.memset(e3[0:64, 0, :], 1.0)            # top pad row
                    nc.gpsimd.memset(e3[0:64, H + 1, :], 1.0)        # bottom pad
                    nc.gpsimd.memset(e3[0:64, 1:H + 1, 0], 1.0)      # left col
                    nc.gpsimd.memset(e3[0:64, 1:H + 1, W + 1], 1.0)  # right col
                    # half1 row 65 + all of region1 half1 are read only
                    # against phantom zero weights: just keep them finite
                    nc.gpsimd.memset(e3[64:128, H + 1, :], 1.0)
                    nc.gpsimd.memset(et[64:128, PAD_TOT:PAD_TOT + H * PAD_W],
                                     1.0)
                et = ets[bufslot]
                et3 = et.rearrange("p (h w) -> p h w", w=PAD_W)
                # region0: half0 (partitions 0-63) = padded exp(x); half1 =
                # same shifted down one padded row. region1 (free offset
                # +PAD_TOT): half0 = E shifted down two rows.
                # band-wise prep: each act half is chased immediately by its
                # dup (half1 = E+1row) and region1 (= E+2rows) copy bands so
                # early strips' matmuls unlock while the second half runs.
                nc.scalar.activation(et3[0:CIN, 1:HH + 1, 1:W + 1],
                                     xt3[:, 0:HH, :], Exp)
                nc.sync.dma_start(et[64:128, 0:HH * PAD_W],
                                  et[0:64, PAD_W:(HH + 1) * PAD_W])
                nc.sync.dma_start(et[0:64, PAD_TOT:PAD_TOT + (HH - 1) * PAD_W],
                                  et[0:64, 2 * PAD_W:(HH + 1) * PAD_W])
                nc.scalar.activation(et3[0:CIN, HH + 1:H + 1, 1:W + 1],
                                     xt3[:, HH:H, :], Exp)
                nc.sync.dma_start(et[64:128, HH * PAD_W:PAD_TOT - PAD_W],
                                  et[0:64, (HH + 1) * PAD_W:PAD_TOT])
                nc.sync.dma_start(
                    et[0:64, PAD_TOT + (HH - 1) * PAD_W:PAD_TOT + H * PAD_W],
                    et[0:64, (HH + 1) * PAD_W:PAD_TOT])

                # one PSUM tile per STRIP-PAIR (2 banks); each matmul still
                # targets a single bank-sized slice
                ptp = {p: pp.tile([COUT, 2 * SPB], f32,
                                  tag=f"pt{p}", bufs=1, name=f"pt_{i}_{p}")
                       for p in range(NTILES // 2)}
                for g in range(3):
                    for t in range(NTILES):
                        sl = ptp[t // 2][:, (t % 2) * SPB:(t % 2 + 1) * SPB]
                        nc.tensor.matmul(
                            sl, wdr4[:, g, :, :], make_rhs(et, t, g),
                            start=(g == 0), stop=(g == 2), perf_mode=DR)
                        if g == 2:
                            epilogue(i, b, t, ptp[t // 2])
    nc.compile()
    return nc


def _prep_weights(weight, bias):
    """wdr[p, g, k, o]: DoubleRow stationary layout, 3 groups x 2 k-tiles.
    partition p<64 -> half0 tap weight exp(w[o, p, tap0]); p>=64 -> half1
    tap (or 0 for phantom k-tiles)."""
    import ml_dtypes
    ew = np.exp(weight.astype(np.float32))           # [COUT, CIN, 3, 3]
    wdr = np.zeros((128, 3, 2, COUT), np.float32)
    for g in range(3):
        for k in range(2):
            tap0, tap1 = GROUP_TAPS[g][k]
            wdr[0:64, g, k, :] = ew[:, :, tap0[0], tap0[1]].T
            if tap1 is not None:
                wdr[64:128, g, k, :] = ew[:, :, tap1[0], tap1[1]].T
    wdr = wdr.reshape(128, 3 * 2 * COUT).astype(ml_dtypes.float8_e4m3)
    bvec = (KSUM + bias.astype(np.float32)).reshape(COUT, 1)
    return wdr, bvec


def _in_map(x_core, weight, bias):
    """Build the per-core input map. x_core: [BL, CIN, H, W] float32."""
    import ml_dtypes
    wdr, bvec = _prep_weights(weight, bias)
    x_bf = np.ascontiguousarray(x_core.astype(ml_dtypes.bfloat16))
    return {"x": x_bf, "wdr": wdr, "bvec": bvec}


def kernel(x, weight, bias):
    from concourse import bass_utils

    x = np.ascontiguousarray(np.asarray(x, dtype=np.float32))
    weight = np.asarray(weight, dtype=np.float32)
    bias = np.asarray(bias, dtype=np.float32)

    if "nc" not in _CACHE:
        _CACHE["nc"] = _build()
    nc = _CACHE["nc"]

    in_maps = [_in_map(x[c * BL:(c + 1) * BL], weight, bias)
               for c in range(NCORES)]
    res = bass_utils.run_bass_kernel_spmd(nc, in_maps, core_ids=list(range(NCORES)))
    return np.concatenate([np.asarray(r["out"], dtype=np.float32)
                           for r in res.results], axis=0)


# revision 16
# speedup vs baseline: 1.3766x; 1.3766x over previous
"""Trainium2 Bass kernel for nn_CustomConv2d_32538672234916.

out[b,o,h,w] = K - sum_{ci,kh,kw} exp(x_patch)*exp(w) + bias[o],  K = Cin*kh*kw = 576
i.e. out = (K + bias) - conv2d(exp(x) [1-padded], exp(weight), stride 1)

Sharding: data-parallel over batch B=16 across 8 cores (2 batches/core),
weights/bias replicated.

Per-core GEMM formulation (fp8 DoubleRow): SBUF holds exp(x) in fp8 three
times: region0 half0 (partitions 0-63) = padded exp(x) "E", region0 half1
(partitions 64-127) = E shifted down one padded row, region1 (free offset
+PAD_TOT) half0 = E shifted down two rows (region1 half1 is only ever
multiplied by phantom zero weights). Each 8-row output strip (512 px)
accumulates THREE MatmulPerfMode.DoubleRow fp8 matmuls (0.5 PE-cycles/row;
k-tile-dim stride = PAD_TOT, nested/non-overlapping -- the hw AP walker
rejects overlapping k-tile strides). Matmul g=kw covers the tap column:
  ktile0 -> taps (0,kw) [half0] + (1,kw) [half1]
  ktile1 -> taps (2,kw) [region1 half0] + phantom w=0 [region1 half1]
Tensor time ~= 16 strips * 3 * 256 cyc = 5.1us/core.

IO: x uploaded bf16 (1MB/core), exp runs bf16->fp8 on the Act engine, out
written bf16 (2MB/core) and widened to f32 on host. Epilogue (PSUM f32 *-1
+ (K+bias) -> bf16) alternates DVE / GpSimd so neither engine is the wall.
fp8 end-to-end rel err ~7e-3 vs the 2e-2 gate.
"""
import sys
sys.path.insert(0, '/opt/trn_rl_repo')
import numpy as np

B, CIN, H, W = 16, 64, 64, 64
COUT = 128
NCORES = 8
BL = B // NCORES          # batches per core
PAD_W = W + 2             # 66
PAD_TOT = PAD_W * (H + 2) # 66*66 = 4356
KSUM = float(CIN * 9)     # 576
ROWS_PER_TILE = 8
NTILES = H // ROWS_PER_TILE  # 8 strips per image
HH = H // 2

# taps covered: [(half0 tap, half1 tap), ...] per (group=kw, ktile);
# ktile0 reads region0 = (E, E+1row), ktile1 reads region1 = (E+2rows, any)
# at k-tile-dim stride PAD_TOT; None = phantom zero weight.
GROUP_TAPS = [
    [((0, 0), (1, 0)), ((2, 0), None)],
    [((0, 1), (1, 1)), ((2, 1), None)],
    [((0, 2), (1, 2)), ((2, 2), None)],
]

_CACHE = {}

# x upload dtype: bf16 (safe) or f8 (halves x HBM/DMA traffic; total rel err
# ~1.3e-2 vs the 2e-2 gate instead of ~7e-3)
XDT = "bf16"


def _build(reps=1):
    from concourse import bacc, mybir
    from concourse.tile import TileContext

    f32 = mybir.dt.float32
    bf16 = mybir.dt.bfloat16
    f8 = mybir.dt.float8e4
    Exp = mybir.ActivationFunctionType.Exp
    DR = mybir.MatmulPerfMode.DoubleRow

    nc = bacc.Bacc("TRN2", target_bir_lowering=False, debug=False)
    xdt = {"bf16": bf16, "f8": f8}[XDT]
    x_d = nc.dram_tensor("x", [BL, CIN, H, W], xdt, kind="ExternalInput")
    wdr_d = nc.dram_tensor("wdr", [128, 3 * 2 * COUT], f8, kind="ExternalInput")
    bvec_d = nc.dram_tensor("bvec", [COUT, 1], f32, kind="ExternalInput")
    out_d = nc.dram_tensor("out", [BL, COUT, H, W], bf16, kind="ExternalOutput")
    x_ap = x_d.ap()
    out_ap = out_d.ap()

    with TileContext(nc) as tc:
        with tc.tile_pool(name="consts", bufs=1) as consts, \
             tc.tile_pool(name="xp", bufs=2) as xp, \
             tc.tile_pool(name="ep", bufs=2) as ep, \
             tc.tile_pool(name="rp", bufs=4) as rp, \
             tc.tile_pool(name="pp", bufs=1, space="PSUM") as pp:
            wdr_t = consts.tile([128, 3 * 2 * COUT], f8)
            bv_t = consts.tile([COUT, 1], f32)
            wdr4 = wdr_t.rearrange("p (g k m) -> p g k m", g=3, k=2)

            def load_consts():
                nc.sync.dma_start(wdr_t[:], wdr_d.ap())
                nc.sync.dma_start(bv_t[:], bvec_d.ap())

            # out-DMA granularity: OUTG strips per transfer. Fewer DMA
            # instructions = less (shared) descriptor-gen serialization.
            OUTG = 4
            SPB = ROWS_PER_TILE * W
            epi_state = {}

            def epilogue(i, b, t, pt):
                # Evacuate a PAIR of strips (one PSUM tile spanning 2 banks)
                # per instruction: PSUM f32 * -1 + (K+bias) -> bf16. GPSIMD
                # can't read PSUM on this target, so pairs split between DVE
                # (7/8) and the Act engine (1/8; Identity shares the loaded
                # act table with Exp). Fires on the pair's second strip.
                if t % 2 == 0:
                    return
                slot = t % OUTG
                if slot == 1:
                    epi_state["res"] = rp.tile([COUT, OUTG * SPB], bf16,
                                               tag="res", name=f"res_{b}_{t}")
                res = epi_state["res"]
                dst = res[:, (slot - 1) * SPB:(slot + 1) * SPB]
                on_act = (t == 3 and i % 2 == 0)
                if on_act:
                    nc.scalar.activation(dst, pt[:],
                                         mybir.ActivationFunctionType.Identity,
                                         bias=bv_t[:], scale=-1.0)
                else:
                    nc.vector.tensor_scalar(dst, pt[:], -1.0, bv_t[:],
                                            mybir.AluOpType.mult,
                                            mybir.AluOpType.add)
                if slot == OUTG - 1:
                    # out-DMA via the (otherwise idle) gpsimd SWDGE ring:
                    # off the SP queue and off the shared HWDGE
                    t0 = t - (OUTG - 1)
                    nc.gpsimd.dma_start(
                        out_ap[b][:, t0 * ROWS_PER_TILE:(t + 1) * ROWS_PER_TILE, :],
                        res[:])

            def make_rhs(et, t, kw):
                # [p, 2(k-tile, stride PAD_TOT), 8 rows, 64 cols] — nested
                # non-overlapping strides (hw rejects overlapping k-tile APs)
                e4 = et.rearrange("p (r h w) -> p r h w", r=2, w=PAD_W)
                h0 = t * ROWS_PER_TILE
                return e4[0:128, :, h0:h0 + ROWS_PER_TILE, kw:kw + W]

            ets = {}
            for i, b in enumerate([b for _ in range(reps) for b in range(BL)]):
                xt = xp.tile([CIN, H * W], xdt, tag="xt", name=f"xt_{i}")
                nc.sync.dma_start(xt[:], x_ap[b].rearrange("c h w -> c (h w)"))
                if i == 0:
                    # after the first x DMA so x-data flows immediately
                    load_consts()
                xt3 = xt.rearrange("p (h w) -> p h w", w=W)
                bufslot = i % 2
                if bufslot not in ets:
                    # pad cells are written once per buffer and persist
                    # (later iterations only rewrite the interiors)
                    et = ep.tile([128, 2 * PAD_TOT], f8, tag="et",
                                 name=f"et_{bufslot}")
                    ets[bufslot] = et
                    e3 = et.rearrange("p (h w) -> p h w", w=PAD_W)
                    nc.gpsimd.memset(e3[0:64, 0, :], 1.0)            # top pad row
                    nc.gpsimd.memset(e3[0:64, H + 1, :], 1.0)        # bottom pad
                    nc.gpsimd.memset(e3[0:64, 1:H + 1, 0], 1.0)      # left col
                    nc.gpsimd.memset(e3[0:64, 1:H + 1, W + 1], 1.0)  # right col
                    # half1 row 65 + all of region1 half1 are read only
                    # against phantom zero weights: just keep them finite
                    nc.gpsimd.memset(e3[64:128, H + 1, :], 1.0)
                    nc.gpsimd.memset(et[64:128, PAD_TOT:PAD_TOT + H * PAD_W],
                                     1.0)
                et = ets[bufslot]
                et3 = et.rearrange("p (h w) -> p h w", w=PAD_W)
                # region0: half0 (partitions 0-63) = padded exp(x); half1 =
                # same shifted down one padded row. region1 (free offset
                # +PAD_TOT): half0 = E shifted down two rows.
                # band-wise prep: each act half is chased immediately by its
                # dup (half1 = E+1row) and region1 (= E+2rows) copy bands so
                # early strips' matmuls unlock while the second half runs.
                nc.scalar.activation(et3[0:CIN, 1:HH + 1, 1:W + 1],
                                     xt3[:, 0:HH, :], Exp)
                nc.sync.dma_start(et[64:128, 0:HH * PAD_W],
                                  et[0:64, PAD_W:(HH + 1) * PAD_W])
                nc.sync.dma_start(et[0:64, PAD_TOT:PAD_TOT + (HH - 1) * PAD_W],
                                  et[0:64, 2 * PAD_W:(HH + 1) * PAD_W])
                nc.scalar.activation(et3[0:CIN, HH + 1:H + 1, 1:W + 1],
                                     xt3[:, HH:H, :], Exp)
                nc.sync.dma_start(et[64:128, HH * PAD_W:PAD_TOT - PAD_W],
                                  et[0:64, (HH + 1) * PAD_W:PAD_TOT])
                nc.sync.dma_start(
                    et[0:64, PAD_TOT + (HH - 1) * PAD_W:PAD_TOT + H * PAD_W],
                    et[0:64, (HH + 1) * PAD_W:PAD_TOT])

                # one PSUM tile per STRIP-PAIR (2 banks); each matmul still
                # targets a single bank-sized slice
                ptp = {p: pp.tile([COUT, 2 * SPB], f32,
                                  tag=f"pt{p}", bufs=1, name=f"pt_{i}_{p}")
                       for p in range(NTILES // 2)}
                for g in range(3):
                    for t in range(NTILES):
                        sl = ptp[t // 2][:, (t % 2) * SPB:(t % 2 + 1) * SPB]
                        nc.tensor.matmul(
                            sl, wdr4[:, g, :, :], make_rhs(et, t, g),
                            start=(g == 0), stop=(g == 2), perf_mode=DR)
                        if g == 2:
                            epilogue(i, b, t, ptp[t // 2])
    nc.compile()
    return nc


def _prep_weights(weight, bias):
    """wdr[p, g, k, o]: DoubleRow stationary layout, 3 groups x 2 k-tiles.
    partition p<64 -> half0 tap weight exp(w[o, p, tap0]); p>=64 -> half1
    tap (or 0 for phantom k-tiles)."""
    import ml_dtypes
    ew = np.exp(weight.astype(np.float32))           # [COUT, CIN, 3, 3]
    wdr = np.zeros((128, 3, 2, COUT), np.float32)
    for g in range(3):
        for k in range(2):
            tap0, tap1 = GROUP_TAPS[g][k]
            wdr[0:64, g, k, :] = ew[:, :, tap0[0], tap0[1]].T
            if tap1 is not None:
                wdr[64:128, g, k, :] = ew[:, :, tap1[0], tap1[1]].T
    wdr = wdr.reshape(128, 3 * 2 * COUT).astype(ml_dtypes.float8_e4m3)
    bvec = (KSUM + bias.astype(np.float32)).reshape(COUT, 1)
    return wdr, bvec


def _in_map(x_core, weight, bias):
    """Build the per-core input map. x_core: [BL, CIN, H, W] float32."""
    import ml_dtypes
    wdr, bvec = _prep_weights(weight, bias)
    xdt = {"bf16": ml_dtypes.bfloat16, "f8": ml_dtypes.float8_e4m3}[XDT]
    x_q = np.ascontiguousarray(x_core.astype(xdt))
    return {"x": x_q, "wdr": wdr, "bvec": bvec}


def kernel(x, weight, bias):
    from concourse import bass_utils

    x = np.ascontiguousarray(np.asarray(x, dtype=np.float32))
    weight = np.asarray(weight, dtype=np.float32)
    bias = np.asarray(bias, dtype=np.float32)

    if "nc" not in _CACHE:
        _CACHE["nc"] = _build()
    nc = _CACHE["nc"]

    in_maps = [_in_map(x[c * BL:(c + 1) * BL], weight, bias)
               for c in range(NCORES)]
    res = bass_utils.run_bass_kernel_spmd(nc, in_maps, core_ids=list(range(NCORES)))
    return np.concatenate([np.asarray(r["out"], dtype=np.float32)
                           for r in res.results], axis=0)


# revision 18
# speedup vs baseline: 1.5517x; 1.1272x over previous
"""Trainium2 Bass kernel for nn_CustomConv2d_32538672234916.

out[b,o,h,w] = K - sum_{ci,kh,kw} exp(x_patch)*exp(w) + bias[o],  K = Cin*kh*kw = 576
i.e. out = (K + bias) - conv2d(exp(x) [1-padded], exp(weight), stride 1)

Sharding: data-parallel over batch B=16 across 8 cores (2 batches/core),
weights/bias replicated.

Per-core GEMM formulation (fp8 DoubleRow): SBUF holds exp(x) in fp8 three
times: region0 half0 (partitions 0-63) = padded exp(x) "E", region0 half1
(partitions 64-127) = E shifted down one padded row, region1 (free offset
+PAD_TOT) half0 = E shifted down two rows (region1 half1 is only ever
multiplied by phantom zero weights). Each 8-row output strip (512 px)
accumulates THREE MatmulPerfMode.DoubleRow fp8 matmuls (0.5 PE-cycles/row;
k-tile-dim stride = PAD_TOT, nested/non-overlapping -- the hw AP walker
rejects overlapping k-tile strides). Matmul g=kw covers the tap column:
  ktile0 -> taps (0,kw) [half0] + (1,kw) [half1]
  ktile1 -> taps (2,kw) [region1 half0] + phantom w=0 [region1 half1]
Tensor time ~= 16 strips * 3 * 256 cyc = 5.1us/core.

IO: x uploaded bf16-or-fp8 (XDT; 1 or 0.5 MB/core), exp runs on the Act
engine straight into the fp8 tap layout (band-wise, each act half chased by
its dup/region copy DMAs so early strips unlock), out written bf16
(2MB/core) and widened to f32 on host. Epilogue evacuates PSUM strip-PAIRS
(f32 * -1 + (K+bias) -> bf16), split 7/8 DVE + 1/8 Act (Identity shares the
loaded act table with Exp; GPSIMD cannot read PSUM on this target); out-DMAs
ride the idle gpsimd SWDGE ring. Steady state ~8.6us/core vs 17.8us for the
bf16 f32-IO baseline.
"""
import sys
sys.path.insert(0, '/opt/trn_rl_repo')
import numpy as np

B, CIN, H, W = 16, 64, 64, 64
COUT = 128
NCORES = 8
BL = B // NCORES          # batches per core
PAD_W = W + 2             # 66
PAD_TOT = PAD_W * (H + 2) # 66*66 = 4356
KSUM = float(CIN * 9)     # 576
ROWS_PER_TILE = 8
NTILES = H // ROWS_PER_TILE  # 8 strips per image
HH = H // 2

# taps covered: [(half0 tap, half1 tap), ...] per (group=kw, ktile);
# ktile0 reads region0 = (E, E+1row), ktile1 reads region1 = (E+2rows, any)
# at k-tile-dim stride PAD_TOT; None = phantom zero weight.
GROUP_TAPS = [
    [((0, 0), (1, 0)), ((2, 0), None)],
    [((0, 1), (1, 1)), ((2, 1), None)],
    [((0, 2), (1, 2)), ((2, 2), None)],
]

_CACHE = {}

# x upload dtype: bf16 (safe) or f8 (halves x HBM/DMA traffic; total rel err
# ~1.3e-2 vs the 2e-2 gate instead of ~7e-3)
XDT = "f8"


def _build(reps=1):
    from concourse import bacc, mybir
    from concourse.tile import TileContext

    f32 = mybir.dt.float32
    bf16 = mybir.dt.bfloat16
    f8 = mybir.dt.float8e4
    Exp = mybir.ActivationFunctionType.Exp
    DR = mybir.MatmulPerfMode.DoubleRow

    nc = bacc.Bacc("TRN2", target_bir_lowering=False, debug=False)
    xdt = {"bf16": bf16, "f8": f8}[XDT]
    x_d = nc.dram_tensor("x", [BL, CIN, H, W], xdt, kind="ExternalInput")
    wdr_d = nc.dram_tensor("wdr", [128, 3 * 2 * COUT], f8, kind="ExternalInput")
    bvec_d = nc.dram_tensor("bvec", [COUT, 1], f32, kind="ExternalInput")
    out_d = nc.dram_tensor("out", [BL, COUT, H, W], bf16, kind="ExternalOutput")
    x_ap = x_d.ap()
    out_ap = out_d.ap()

    with TileContext(nc) as tc:
        with tc.tile_pool(name="consts", bufs=1) as consts, \
             tc.tile_pool(name="xp", bufs=2) as xp, \
             tc.tile_pool(name="ep", bufs=2) as ep, \
             tc.tile_pool(name="rp", bufs=4) as rp, \
             tc.tile_pool(name="pp", bufs=1, space="PSUM") as pp:
            wdr_t = consts.tile([128, 3 * 2 * COUT], f8)
            bv_t = consts.tile([COUT, 1], f32)
            wdr4 = wdr_t.rearrange("p (g k m) -> p g k m", g=3, k=2)

            def load_consts():
                nc.sync.dma_start(wdr_t[:], wdr_d.ap())
                nc.sync.dma_start(bv_t[:], bvec_d.ap())

            # out-DMA granularity: OUTG strips per transfer. Fewer DMA
            # instructions = less (shared) descriptor-gen serialization.
            OUTG = 4
            SPB = ROWS_PER_TILE * W
            epi_state = {}

            def epilogue(i, b, t, pt):
                # Evacuate a PAIR of strips (one PSUM tile spanning 2 banks)
                # per instruction: PSUM f32 * -1 + (K+bias) -> bf16. GPSIMD
                # can't read PSUM on this target, so pairs split between DVE
                # (7/8) and the Act engine (1/8; Identity shares the loaded
                # act table with Exp). Fires on the pair's second strip.
                if t % 2 == 0:
                    return
                slot = t % OUTG
                if slot == 1:
                    epi_state["res"] = rp.tile([COUT, OUTG * SPB], bf16,
                                               tag="res", name=f"res_{b}_{t}")
                res = epi_state["res"]
                dst = res[:, (slot - 1) * SPB:(slot + 1) * SPB]
                on_act = (t == 3 and i % 2 == 0)
                if on_act:
                    nc.scalar.activation(dst, pt[:],
                                         mybir.ActivationFunctionType.Identity,
                                         bias=bv_t[:], scale=-1.0)
                else:
                    nc.vector.tensor_scalar(dst, pt[:], -1.0, bv_t[:],
                                            mybir.AluOpType.mult,
                                            mybir.AluOpType.add)
                if slot == OUTG - 1:
                    # out-DMA via the (otherwise idle) gpsimd SWDGE ring:
                    # off the SP queue and off the shared HWDGE
                    t0 = t - (OUTG - 1)
                    nc.gpsimd.dma_start(
                        out_ap[b][:, t0 * ROWS_PER_TILE:(t + 1) * ROWS_PER_TILE, :],
                        res[:])

            def make_rhs(et, t, kw):
                # [p, 2(k-tile, stride PAD_TOT), 8 rows, 64 cols] — nested
                # non-overlapping strides (hw rejects overlapping k-tile APs)
                e4 = et.rearrange("p (r h w) -> p r h w", r=2, w=PAD_W)
                h0 = t * ROWS_PER_TILE
                return e4[0:128, :, h0:h0 + ROWS_PER_TILE, kw:kw + W]

            ets = {}
            for i, b in enumerate([b for _ in range(reps) for b in range(BL)]):
                xt = xp.tile([CIN, H * W], xdt, tag="xt", name=f"xt_{i}")
                nc.sync.dma_start(xt[:], x_ap[b].rearrange("c h w -> c (h w)"))
                if i == 0:
                    # after the first x DMA so x-data flows immediately
                    load_consts()
                xt3 = xt.rearrange("p (h w) -> p h w", w=W)
                bufslot = i % 2
                if bufslot not in ets:
                    # pad cells are written once per buffer and persist
                    # (later iterations only rewrite the interiors)
                    et = ep.tile([128, 2 * PAD_TOT], f8, tag="et",
                                 name=f"et_{bufslot}")
                    ets[bufslot] = et
                    e3 = et.rearrange("p (h w) -> p h w", w=PAD_W)
                    nc.gpsimd.memset(e3[0:64, 0, :], 1.0)            # top pad row
                    nc.gpsimd.memset(e3[0:64, H + 1, :], 1.0)        # bottom pad
                    nc.gpsimd.memset(e3[0:64, 1:H + 1, 0], 1.0)      # left col
                    nc.gpsimd.memset(e3[0:64, 1:H + 1, W + 1], 1.0)  # right col
                    # half1 row 65 + all of region1 half1 are read only
                    # against phantom zero weights: just keep them finite
                    nc.gpsimd.memset(e3[64:128, H + 1, :], 1.0)
                    nc.gpsimd.memset(et[64:128, PAD_TOT:PAD_TOT + H * PAD_W],
                                     1.0)
                et = ets[bufslot]
                et3 = et.rearrange("p (h w) -> p h w", w=PAD_W)
                # region0: half0 (partitions 0-63) = padded exp(x); half1 =
                # same shifted down one padded row. region1 (free offset
                # +PAD_TOT): half0 = E shifted down two rows.
                # band-wise prep: each act half is chased immediately by its
                # dup (half1 = E+1row) and region1 (= E+2rows) copy bands so
                # early strips' matmuls unlock while the second half runs.
                nc.scalar.activation(et3[0:CIN, 1:HH + 1, 1:W + 1],
                                     xt3[:, 0:HH, :], Exp)
                nc.sync.dma_start(et[64:128, 0:HH * PAD_W],
                                  et[0:64, PAD_W:(HH + 1) * PAD_W])
                nc.sync.dma_start(et[0:64, PAD_TOT:PAD_TOT + (HH - 1) * PAD_W],
                                  et[0:64, 2 * PAD_W:(HH + 1) * PAD_W])
                nc.scalar.activation(et3[0:CIN, HH + 1:H + 1, 1:W + 1],
                                     xt3[:, HH:H, :], Exp)
                nc.sync.dma_start(et[64:128, HH * PAD_W:PAD_TOT - PAD_W],
                                  et[0:64, (HH + 1) * PAD_W:PAD_TOT])
                nc.sync.dma_start(
                    et[0:64, PAD_TOT + (HH - 1) * PAD_W:PAD_TOT + H * PAD_W],
                    et[0:64, (HH + 1) * PAD_W:PAD_TOT])

                # one PSUM tile per STRIP-PAIR (2 banks); each matmul still
                # targets a single bank-sized slice
                ptp = {p: pp.tile([COUT, 2 * SPB], f32,
                                  tag=f"pt{p}", bufs=1, name=f"pt_{i}_{p}")
                       for p in range(NTILES // 2)}
                for g in range(3):
                    for t in range(NTILES):
                        sl = ptp[t // 2][:, (t % 2) * SPB:(t % 2 + 1) * SPB]
                        nc.tensor.matmul(
                            sl, wdr4[:, g, :, :], make_rhs(et, t, g),
                            start=(g == 0), stop=(g == 2), perf_mode=DR)
                        if g == 2:
                            epilogue(i, b, t, ptp[t // 2])
    nc.compile()
    return nc


def _prep_weights(weight, bias):
    """wdr[p, g, k, o]: DoubleRow stationary layout, 3 groups x 2 k-tiles.
    partition p<64 -> half0 tap weight exp(w[o, p, tap0]); p>=64 -> half1
    tap (or 0 for phantom k-tiles)."""
    import ml_dtypes
    ew = np.exp(weight.astype(np.float32))           # [COUT, CIN, 3, 3]
    wdr = np.zeros((128, 3, 2, COUT), np.float32)
    for g in range(3):
        for k in range(2):
            tap0, tap1 = GROUP_TAPS[g][k]
            wdr[0:64, g, k, :] = ew[:, :, tap0[0], tap0[1]].T
            if tap1 is not None:
                wdr[64:128, g, k, :] = ew[:, :, tap1[0], tap1[1]].T
    wdr = wdr.reshape(128, 3 * 2 * COUT).astype(ml_dtypes.float8_e4m3)
    bvec = (KSUM + bias.astype(np.float32)).reshape(COUT, 1)
    return wdr, bvec


def _in_map(x_core, weight, bias):
    """Build the per-core input map. x_core: [BL, CIN, H, W] float32."""
    import ml_dtypes
    wdr, bvec = _prep_weights(weight, bias)
    xdt = {"bf16": ml_dtypes.bfloat16, "f8": ml_dtypes.float8_e4m3}[XDT]
    x_q = np.ascontiguousarray(x_core.astype(xdt))
    return {"x": x_q, "wdr": wdr, "bvec": bvec}


def kernel(x, weight, bias):
    from concourse import bass_utils

    x = np.ascontiguousarray(np.asarray(x, dtype=np.float32))
    weight = np.asarray(weight, dtype=np.float32)
    bias = np.asarray(bias, dtype=np.float32)

    if "nc" not in _CACHE:
        _CACHE["nc"] = _build()
    nc = _CACHE["nc"]

    in_maps = [_in_map(x[c * BL:(c + 1) * BL], weight, bias)
               for c in range(NCORES)]
    res = bass_utils.run_bass_kernel_spmd(nc, in_maps, core_ids=list(range(NCORES)))
    return np.concatenate([np.asarray(r["out"], dtype=np.float32)
                           for r in res.results], axis=0)
